# revision 1
# baseline (speedup 1.0000x reference)
"""AttentionPooling (segment softmax pooling) on 8 Trainium2 NeuronCores.

z[b] = sum_i softmax_within_segment(alpha)_i * x_i  for segment b, where
alpha = tanh(x @ W1.T) @ W2.T.

Strategy (data parallel over segments):
- batch is sorted, B = 1024 = 8 * 128, so core c owns segments
  [128c, 128(c+1)) — a contiguous row range of x. No cross-core segments,
  so the host just concatenates per-core results.
- alpha for this distribution lies in ~[-3, 3], so exp() without the
  per-segment max subtraction is numerically safe; softmax = e / seg_sum(e).
- Per 128-row tile on device:
    yT   = W1 @ x_tile.T          (PE, bf16, K=256 via 2 chunks)
    th   = tanh(yT)               (ACT, PSUM->SBUF bf16)
    a    = th.T @ W2              (PE -> (128 rows x 1) PSUM)
    e    = exp(a)                 (ACT -> e_buf in SBUF, dumped at end)
    E    = (iota == colidx%32) * e (DVE, built for 4 tiles per op via
                                   stride-0 broadcast APs; one-hot is only
                                   32 wide = segment index mod 32)
    gpool += E.T @ x_tile         (PE, (32 x 256) per-group PSUM; a 32-tile
                                   group spans <= ~10 segments so mod-32 is
                                   collision-free within a group)
  ...and once per 32-tile group:
    pool += scatter_g.T @ gpool   (PE, f32 0/1 scatter matmul - exact -
                                   into the persistent (128 segs x 256) pool)
- Host: denominator d_s = segment_sum(bf16(e)) from the e dump (exactly
  the same bf16 values the E matrix used), z = pool / d.

x is shipped twice in bf16 (row-major for pooling, transposed for the
matmul contraction over D) — 2 bytes * 2 orientations = same HBM traffic
as reading the f32 x once; the kernel is HBM-bandwidth bound.
"""

import numpy as np
import ml_dtypes

import concourse.bacc as bacc
import concourse.mybir as mybir
import concourse.tile as tile
from concourse.bass_utils import run_bass_kernel_spmd

bf16 = ml_dtypes.bfloat16
F32 = mybir.dt.float32
BF16 = mybir.dt.bfloat16
AF = mybir.ActivationFunctionType
ALU = mybir.AluOpType

NCORES = 8
D = 256
H = 128
SEGS_PER_CORE = 128
GT = 32          # max tiles per DMA group; a group spans <= ~10 segments
QUAD = 4         # tiles per mm1/psum_y batch (N' = 512)
EW = 32          # one-hot width: local segment index mod EW within a group

_kernel_cache = {}


def _group_plan(nt):
    """Uniform GT-tile DMA groups (irregular group sizes measurably hurt
    the DMA/PE pipeline on hardware)."""
    assert nt % GT == 0
    return [GT] * (nt // GT)


def _build_kernel(nt):
    """Build + compile the per-core SPMD kernel for nt 128-row tiles."""
    assert nt % 8 == 0 and GT % QUAD == 0
    nc = bacc.Bacc("TRN2", target_bir_lowering=False, debug=False)

    x_nat_d = nc.dram_tensor("x_nat", [128, nt, D], BF16, kind="ExternalInput").ap()
    xt_d = nc.dram_tensor("xT", [128, 2, nt * 128], BF16, kind="ExternalInput").ap()
    ci_d = nc.dram_tensor("colidx", [128, nt], BF16, kind="ExternalInput").ap()
    w1t_d = nc.dram_tensor("W1T", [128, 2, H], BF16, kind="ExternalInput").ap()
    w2_d = nc.dram_tensor("W2c", [H, 1], BF16, kind="ExternalInput").ap()
    iota_d = nc.dram_tensor("iota", [128, EW], BF16, kind="ExternalInput").ap()
    sizes = _group_plan(nt)
    ngroups = len(sizes)
    scat_d = nc.dram_tensor("scat", [EW, ngroups, SEGS_PER_CORE], F32,
                            kind="ExternalInput").ap()
    out_d = nc.dram_tensor("out", [SEGS_PER_CORE, D], F32, kind="ExternalOutput").ap()
    e_out_d = nc.dram_tensor("e_out", [128, nt], BF16, kind="ExternalOutput").ap()

    with tile.TileContext(nc) as tc:
        with (
            tc.tile_pool(name="const", bufs=1) as constp,
            tc.tile_pool(name="xn", bufs=3) as xnp,
            tc.tile_pool(name="xt", bufs=3) as xtp,
            tc.tile_pool(name="th", bufs=3) as thp,
            tc.tile_pool(name="ee", bufs=4) as eep,
            tc.tile_pool(name="out", bufs=1) as outp,
            tc.tile_pool(name="psum_y", bufs=2, space="PSUM") as psumy,
            tc.tile_pool(name="psum_al", bufs=1, space="PSUM") as psumal,
            tc.tile_pool(name="psum_gp", bufs=2, space="PSUM") as psumgp,
            tc.tile_pool(name="psum_acc", bufs=1, space="PSUM") as psumacc,
        ):
            w1t_sb = constp.tile([128, 2, H], BF16)
            nc.default_dma_engine.dma_start(w1t_sb[:], w1t_d[:])
            w2_sb = constp.tile([H, 1], BF16)
            nc.default_dma_engine.dma_start(w2_sb[:], w2_d[:])
            iota_sb = constp.tile([128, EW], BF16)
            nc.default_dma_engine.dma_start(iota_sb[:], iota_d[:])
            ci_sb = constp.tile([128, nt], BF16)
            nc.default_dma_engine.dma_start(ci_sb[:], ci_d[:])
            scat_sb = constp.tile([EW, ngroups, SEGS_PER_CORE], F32)
            nc.default_dma_engine.dma_start(scat_sb[:], scat_d[:])
            e_buf = constp.tile([128, nt], BF16)

            pool_ps = psumacc.tile([SEGS_PER_CORE, D], F32)

            gstart = 0
            for g, gsize in enumerate(sizes):
                xn = xnp.tile([128, gsize, D], BF16, tag="xn",
                              padded_shape=[128, GT, D])
                nc.default_dma_engine.dma_start(
                    xn[:], x_nat_d[:, gstart:gstart + gsize, :])
                xt = xtp.tile([128, 2, gsize * 128], BF16, tag="xt",
                              padded_shape=[128, 2, GT * 128])
                nc.default_dma_engine.dma_start(
                    xt[:], xt_d[:, :, gstart * 128:(gstart + gsize) * 128])

                gp_ps = psumgp.tile([EW, D], F32, tag="gp")

                # mm1 W1-chunk-outer per half-group (2 quads = 2 PSUM banks)
                nhalf_quads = 2
                for half in range((gsize // QUAD) // nhalf_quads):
                    y_ps = [psumy.tile([128, QUAD * 128], F32, name=f"y{q}",
                                       tag=f"y{q}")
                            for q in range(nhalf_quads)]
                    for chunk in range(2):
                        for q in range(nhalf_quads):
                            qq = half * nhalf_quads + q
                            nc.tensor.matmul(
                                y_ps[q][:], w1t_sb[:, chunk, :],
                                xt[:, chunk, qq * QUAD * 128:(qq + 1) * QUAD * 128],
                                start=(chunk == 0), stop=(chunk == 1))

                    for q in range(nhalf_quads):
                        qq = half * nhalf_quads + q
                        th = thp.tile([128, QUAD * 128], BF16, tag="th")
                        nc.scalar.activation(th[:], y_ps[q][:], AF.Tanh)

                        al_ps = psumal.tile([128, QUAD], F32, tag="al")
                        for j in range(QUAD):
                            nc.tensor.matmul(al_ps[:, j:j + 1],
                                             th[:, j * 128:(j + 1) * 128],
                                             w2_sb[:], start=True, stop=True)
                        t0 = gstart + qq * QUAD
                        nc.scalar.activation(e_buf[:, t0:t0 + QUAD], al_ps[:], AF.Exp)

                        # one-hot(e-weighted) E for the whole quad in 2 DVE ops
                        # via stride-0 broadcast APs
                        S4 = eep.tile([128, QUAD, EW], BF16, tag="S4")
                        nc.vector.tensor_tensor(
                            S4[:],
                            ci_sb[:, t0:t0 + QUAD].broadcast_to([128, QUAD, EW]),
                            iota_sb[:, None, :].broadcast_to([128, QUAD, EW]),
                            ALU.is_equal)
                        E4 = eep.tile([128, QUAD, EW], BF16, tag="E4")
                        nc.vector.tensor_mul(
                            E4[:], S4[:],
                            e_buf[:, t0:t0 + QUAD].broadcast_to([128, QUAD, EW]))
                        for j in range(QUAD):
                            tg = qq * QUAD + j  # tile index within group
                            nc.tensor.matmul(gp_ps[:], E4[:, j, :], xn[:, tg, :],
                                             start=(tg == 0), stop=(tg == gsize - 1))

                # scatter the group pool into the global per-segment pool
                # (f32 matmul with a 0/1 scatter matrix — exact)
                gp_sb = eep.tile([EW, D], F32, tag="gp_sb")
                nc.scalar.activation(gp_sb[:], gp_ps[:], AF.Copy)
                nc.tensor.matmul(pool_ps[:], scat_sb[:, g, :], gp_sb[:],
                                 start=(g == 0), stop=(g == ngroups - 1))
                gstart += gsize

            pool_sb = outp.tile([SEGS_PER_CORE, D], F32)
            nc.scalar.activation(pool_sb[:], pool_ps[:], AF.Copy)
            nc.default_dma_engine.dma_start(out_d[:], pool_sb[:])
            nc.default_dma_engine.dma_start(e_out_d[:], e_buf[:])

    nc.compile()
    return nc


def _prep_core(x, batch, r0, r1, seg0, nt):
    """Host-side shard prep for one core: rows [r0, r1) own segments
    [seg0, seg0+128). Returns the per-core input map."""
    rows = r1 - r0
    pad_rows = nt * 128

    xb = np.zeros((pad_rows, D), dtype=bf16)
    xb[:rows] = x[r0:r1].astype(bf16)
    # (128, nt, D): partition p holds row t*128 + p
    x_nat = np.ascontiguousarray(xb.reshape(nt, 128, D).transpose(1, 0, 2))

    xtb = np.zeros((2, H, pad_rows), dtype=bf16)
    xtb.reshape(D, pad_rows)[:, :rows] = xb[:rows].T
    xT = np.ascontiguousarray(xtb.transpose(1, 0, 2))  # (128, 2, pad_rows)

    seg_local = np.full(pad_rows, -1, dtype=np.int64)
    seg_local[:rows] = batch[r0:r1] - seg0
    ci = np.where(seg_local < 0, -1.0, seg_local % EW).astype(np.float32)
    colidx = np.ascontiguousarray(ci.reshape(nt, 128).T).astype(bf16)  # (128, nt)

    # scatter matrices: scat[k, g, s] = 1 iff group g's pool row k holds
    # local segment s (k = s mod EW). A group spans <= ~10 consecutive
    # segments, so within a group the mod-EW mapping is collision free.
    sizes = _group_plan(nt)
    scat = np.zeros((EW, len(sizes), SEGS_PER_CORE), dtype=np.float32)
    gstart = 0
    for g, gsize in enumerate(sizes):
        segs = np.unique(seg_local[gstart * 128:(gstart + gsize) * 128])
        segs = segs[segs >= 0]
        assert segs.size <= EW, f"group {g} spans {segs.size} segments > EW"
        scat[segs % EW, g, segs] = 1.0
        gstart += gsize

    return {"x_nat": x_nat, "xT": xT, "colidx": colidx, "scat": scat}


def _shared_inputs(W1, W2):
    w1t = np.ascontiguousarray(
        W1.T.astype(bf16).reshape(2, H, H).transpose(1, 0, 2))  # (128, 2, H)
    w2c = np.ascontiguousarray(W2.reshape(H, 1).astype(bf16))
    iota = np.broadcast_to(
        np.arange(EW, dtype=np.float32), (128, EW)).astype(bf16)
    return {"W1T": w1t, "W2c": w2c, "iota": iota}


def _seg_starts(x, batch):
    s = np.searchsorted(batch, np.arange(0, NCORES * SEGS_PER_CORE + 1, SEGS_PER_CORE))
    s[0], s[-1] = 0, x.shape[0]
    return s


def build_in_maps(x, batch, nt):
    s = _seg_starts(x, batch)
    return [_prep_core(x, batch, int(s[c]), int(s[c + 1]), c * SEGS_PER_CORE, nt)
            for c in range(NCORES)]


def pick_nt(x, batch):
    s = _seg_starts(x, batch)
    nt = int(max(-(-(int(s[c + 1] - s[c])) // 128) for c in range(NCORES)))
    return -(-nt // GT) * GT


def kernel(x, batch, W1, W2, B):
    x = np.asarray(x)
    batch = np.asarray(batch)
    W1 = np.asarray(W1)
    W2 = np.asarray(W2)
    B = int(B)
    assert B == NCORES * SEGS_PER_CORE

    nt = pick_nt(x, batch)
    if nt not in _kernel_cache:
        _kernel_cache[nt] = _build_kernel(nt)
    nc = _kernel_cache[nt]

    shared = _shared_inputs(W1, W2)
    in_maps = build_in_maps(x, batch, nt)
    for m in in_maps:
        m.update(shared)

    res = run_bass_kernel_spmd(nc, in_maps, core_ids=list(range(NCORES)))

    seg_starts = _seg_starts(x, batch)
    z = np.empty((B, D), dtype=np.float32)
    for c in range(NCORES):
        num = res.results[c]["out"]  # (128, D)
        # denominator from the e dump, rounded exactly like the E matrix
        e = res.results[c]["e_out"].T.reshape(-1)  # row t*128+p -> e
        r0, r1 = int(seg_starts[c]), int(seg_starts[c + 1])
        seg_local = (batch[r0:r1] - c * SEGS_PER_CORE).astype(np.int64)
        e_rows = e[:r1 - r0].astype(np.float64)
        den = np.bincount(seg_local, weights=e_rows, minlength=SEGS_PER_CORE)
        den = np.where(den == 0.0, 1.0, den).astype(np.float32)
        z[c * SEGS_PER_CORE:(c + 1) * SEGS_PER_CORE] = num / den[:, None]
    return z



# revision 2
# speedup vs baseline: 1.8183x; 1.8183x over previous
"""AttentionPooling v3: ship x once (natural bf16), transpose on-chip.

z[b] = sum_i softmax_within_segment(alpha)_i * x_i, alpha = tanh(x@W1.T)@W2.T.

vs v1 (which shipped x in BOTH orientations = 64MB/core, DMA-bound at
~200us): ship only x_nat (32MB/core), and build x^T on-chip:

- per 4-tile quad: PE-transposes x_nat 128x128 blocks into PSUM (bf16),
  then DVE/ACT copy them to SBUF (alternating, to balance engine load).
  mm1 (y^T = W1 @ x^T) consumes that SBUF copy exactly like v1.
- pooling is the v1 one-hot scheme but FLIPPED: gp^T[d, ew] += x_chunk^T @ E
  with x_nat as the stationary operand (out free size 32 instead of 256).
  Per group: gp^T -> (ACT evac) -> PE transpose -> gp[ew, d] -> f32 scatter
  matmul into the persistent per-segment pool (exact, as v1).
- exp batched per group (1 ACT op), E-build batched per group (2 DVE ops).
- host: denominators from the e_buf dump, z = pool / den (same as v1).

Engine budget per core (sim): DMA ~97us (32MB), PE ~135us (transposes 53
+ mm1 53 + pool 13 + misc), ACT ~95 + evac share, DVE ~37 + evac share.
"""

import numpy as np
import ml_dtypes

import concourse.bacc as bacc
import concourse.mybir as mybir
import concourse.tile as tile
from concourse.bass_utils import run_bass_kernel_spmd

bf16 = ml_dtypes.bfloat16
F32 = mybir.dt.float32
BF16 = mybir.dt.bfloat16
AF = mybir.ActivationFunctionType
ALU = mybir.AluOpType

NCORES = 8
D = 256
H = 128
SEGS_PER_CORE = 128
GT = 32          # tiles per group (one-hot width EW must cover its segments)
EW = 32
QUAD = 4         # tiles per mm1/psum batch

# every XBAR_EVERY-th quad is transposed by the DMA xbar instead of the PE
# (disabled: per-block issue overhead ~650ns dwarfs the 420ns PE saving, and
# the issuing engine's SEQ blocks on the transpose's data waits)
XBAR_EVERY = 10 ** 9
XBAR_PHASE = 3

_kernel_cache = {}


def _group_plan(nt):
    # small leading groups so exp/E-build/pool start early (shorter
    # dependency ladder during pipeline fill), then full GT groups
    sizes = []
    left = nt
    for s in (8, 8, 16):
        if left >= s:
            sizes.append(s)
            left -= s
    while left > 0:
        sizes.append(min(GT, left))
        left -= GT
    return sizes


def _build_kernel(nt):
    assert nt % QUAD == 0
    sizes = _group_plan(nt)
    ngroups = len(sizes)

    nc = bacc.Bacc("TRN2", target_bir_lowering=False, debug=False)

    xn_d = nc.dram_tensor("x_nat", [128, nt, D], BF16, kind="ExternalInput").ap()
    ci_d = nc.dram_tensor("colidx", [128, nt], BF16, kind="ExternalInput").ap()
    w1t_d = nc.dram_tensor("W1T", [128, 2, H], BF16, kind="ExternalInput").ap()
    w2_d = nc.dram_tensor("W2c", [H, 1], BF16, kind="ExternalInput").ap()
    iota_d = nc.dram_tensor("iota", [128, EW], BF16, kind="ExternalInput").ap()
    ident_d = nc.dram_tensor("ident", [128, 128], BF16, kind="ExternalInput").ap()
    identf_d = nc.dram_tensor("identf", [128, 128], F32, kind="ExternalInput").ap()
    scat_d = nc.dram_tensor("scat", [EW, ngroups, SEGS_PER_CORE], F32,
                            kind="ExternalInput").ap()
    out_d = nc.dram_tensor("out", [SEGS_PER_CORE, D], F32, kind="ExternalOutput").ap()
    e_out_d = nc.dram_tensor("e_out", [128, nt], BF16, kind="ExternalOutput").ap()

    with tile.TileContext(nc) as tc:
        with (
            tc.tile_pool(name="const", bufs=1) as constp,
            tc.tile_pool(name="xn", bufs=3) as xnp,
            tc.tile_pool(name="xt", bufs=4) as xtp,
            tc.tile_pool(name="th", bufs=3) as thp,
            tc.tile_pool(name="e4", bufs=2) as e4p,
            tc.tile_pool(name="gps", bufs=2) as gpsp,
            tc.tile_pool(name="out", bufs=1) as outp,
            tc.tile_pool(name="psum_y", bufs=2, space="PSUM") as psumy,
            tc.tile_pool(name="psum_xt", bufs=2, space="PSUM") as psumxt,
            tc.tile_pool(name="psum_al", bufs=1, space="PSUM") as psumal,
            tc.tile_pool(name="psum_gp", bufs=1, space="PSUM") as psumgp,
            tc.tile_pool(name="psum_gt", bufs=1, space="PSUM") as psumgt,
            tc.tile_pool(name="psum_acc", bufs=1, space="PSUM") as psumacc,
        ):
            # first group's x is loaded per-quad ahead of the other consts so
            # the PE can start transposing ~2us in instead of ~14us
            ident_sb = constp.tile([128, 128], BF16)
            nc.sync.dma_start(ident_sb[:], ident_d[:])
            g0size = sizes[0]
            xn0 = xnp.tile([128, g0size, D], BF16, tag="xn",
                           padded_shape=[128, GT, D])
            nc.sync.dma_start(xn0[:, 0:QUAD, :], xn_d[:, 0:QUAD, :])
            w1t_sb = constp.tile([128, 2, H], BF16)
            nc.sync.dma_start(w1t_sb[:], w1t_d[:])
            w2_sb = constp.tile([H, 1], BF16)
            nc.sync.dma_start(w2_sb[:], w2_d[:])
            for q0 in range(QUAD, g0size, QUAD):
                qn = min(QUAD, g0size - q0)
                nc.sync.dma_start(xn0[:, q0:q0 + qn, :],
                                  xn_d[:, q0:q0 + qn, :])
            iota_sb = constp.tile([128, EW], BF16)
            nc.sync.dma_start(iota_sb[:], iota_d[:])
            identf_sb = constp.tile([128, 128], F32)
            nc.sync.dma_start(identf_sb[:], identf_d[:])
            ci_sb = constp.tile([128, nt], BF16)
            nc.sync.dma_start(ci_sb[:], ci_d[:])
            scat_sb = constp.tile([EW, ngroups, SEGS_PER_CORE], F32)
            nc.sync.dma_start(scat_sb[:], scat_d[:])
            e_buf = constp.tile([128, nt], BF16)

            pool_ps = psumacc.tile([SEGS_PER_CORE, D], F32)

            # deferred pooling closures, run one group behind compute
            pending_pool = []

            def emit_pool(g, gsize, gstart, xn, e4g):
                gpt_ps = psumgt.tile([128, 2, EW], F32, tag="gpt")
                for c in range(2):
                    for tg in range(gsize):
                        nc.tensor.matmul(
                            gpt_ps[:, c, :],
                            xn[:, tg, c * 128:(c + 1) * 128],
                            e4g[:, tg, :],
                            start=(tg == 0), stop=(tg == gsize - 1))
                gpt_sb = gpsp.tile([128, 2, EW], F32, tag="gpt_sb")
                nc.scalar.activation(gpt_sb[:], gpt_ps[:], AF.Copy)
                gp_ps = psumgp.tile([EW, D], F32, tag="gp")
                for c in range(2):
                    nc.tensor.transpose(gp_ps[:, c * 128:(c + 1) * 128],
                                        gpt_sb[:, c, :], identf_sb[:])
                gp_sb = gpsp.tile([EW, D], F32, tag="gp_sb")
                nc.scalar.activation(gp_sb[:], gp_ps[:], AF.Copy)
                nc.tensor.matmul(pool_ps[:], scat_sb[:, g, :], gp_sb[:],
                                 start=(g == 0), stop=(g == ngroups - 1))

            gstart = 0
            evac_i = 0
            for g, gsize in enumerate(sizes):
                if g == 0:
                    xn = xn0
                else:
                    xn = xnp.tile([128, gsize, D], BF16, tag="xn",
                                  padded_shape=[128, GT, D])
                    if gstart < 128:
                        # fill phase: halve the transfers so the PE can start
                        # on the group sooner
                        h = (gsize // 2 + QUAD - 1) // QUAD * QUAD
                        nc.sync.dma_start(xn[:, 0:h, :],
                                          xn_d[:, gstart:gstart + h, :])
                        nc.sync.dma_start(xn[:, h:gsize, :],
                                          xn_d[:, gstart + h:gstart + gsize, :])
                    else:
                        nc.sync.dma_start(xn[:],
                                          xn_d[:, gstart:gstart + gsize, :])

                al_ps = psumal.tile([128, gsize], F32, tag="al",
                                    padded_shape=[128, GT])

                for q0 in range(0, gsize, QUAD):
                    qn = min(QUAD, gsize - q0)
                    xt_sb = xtp.tile([128, 2, qn * 128], BF16, tag="xt_sb",
                                     padded_shape=[128, 2, QUAD * 128])
                    if evac_i % XBAR_EVERY == XBAR_PHASE and qn == QUAD:
                        # xbar-DMA transpose straight to SBUF (no PSUM, no
                        # evac); issued on the ACT/DVE queues to keep the SP
                        # sequencer free
                        eng = nc.scalar if (evac_i // XBAR_EVERY) % 2 == 0 \
                            else nc.sync
                        for j in range(qn):
                            t = q0 + j
                            for c in range(2):
                                eng.dma_start_transpose(
                                    xt_sb[:, c, j * 128:(j + 1) * 128],
                                    xn[:, t, c * 128:(c + 1) * 128])
                    else:
                        # PE transpose into PSUM (bf16), evac via DVE
                        xt_ps = psumxt.tile([128, 2, qn * 128], BF16,
                                            tag="xt_ps",
                                            padded_shape=[128, 2, QUAD * 128])
                        for j in range(qn):
                            t = q0 + j
                            for c in range(2):
                                nc.tensor.transpose(
                                    xt_ps[:, c, j * 128:(j + 1) * 128],
                                    xn[:, t, c * 128:(c + 1) * 128],
                                    ident_sb[:])
                        nc.vector.tensor_copy(xt_sb[:], xt_ps[:])
                    evac_i += 1

                    # mm1: y^T = W1c @ x^T  (accumulate over 2 chunks)
                    y_ps = psumy.tile([128, qn * 128], F32, tag="y",
                                      padded_shape=[128, QUAD * 128])
                    for c in range(2):
                        nc.tensor.matmul(y_ps[:], w1t_sb[:, c, :],
                                         xt_sb[:, c, :],
                                         start=(c == 0), stop=(c == 1))
                    th = thp.tile([128, qn * 128], BF16, tag="th",
                                  padded_shape=[128, QUAD * 128])
                    nc.scalar.activation(th[:], y_ps[:], AF.Tanh)
                    for j in range(qn):
                        nc.tensor.matmul(al_ps[:, q0 + j:q0 + j + 1],
                                         th[:, j * 128:(j + 1) * 128],
                                         w2_sb[:], start=True, stop=True)

                nc.scalar.activation(e_buf[:, gstart:gstart + gsize], al_ps[:],
                                     AF.Exp)

                # one-hot E (weighted by e) for the whole group: 2 DVE ops
                s4 = e4p.tile([128, gsize, EW], BF16, tag="s4",
                              padded_shape=[128, GT, EW])
                nc.vector.tensor_tensor(
                    s4[:],
                    ci_sb[:, gstart:gstart + gsize].broadcast_to(
                        [128, gsize, EW]),
                    iota_sb[:, None, :].broadcast_to([128, gsize, EW]),
                    ALU.is_equal)
                e4g = e4p.tile([128, gsize, EW], BF16, tag="e4g",
                               padded_shape=[128, GT, EW])
                nc.vector.tensor_mul(
                    e4g[:], s4[:],
                    e_buf[:, gstart:gstart + gsize].broadcast_to(
                        [128, gsize, EW]))

                # run the PREVIOUS group's pooling now (keeps PE from
                # stalling on this group's E4)
                for fn in pending_pool:
                    fn()
                pending_pool = [
                    (lambda g=g, gsize=gsize, gstart=gstart, xn=xn, e4g=e4g:
                     emit_pool(g, gsize, gstart, xn, e4g))]
                gstart += gsize

            for fn in pending_pool:
                fn()

            pool_sb = outp.tile([SEGS_PER_CORE, D], F32)
            nc.scalar.activation(pool_sb[:], pool_ps[:], AF.Copy)
            nc.sync.dma_start(out_d[:], pool_sb[:])
            nc.sync.dma_start(e_out_d[:], e_buf[:])

    nc.compile()
    return nc


def _prep_core(x, batch, r0, r1, seg0, nt):
    rows = r1 - r0
    pad_rows = nt * 128

    xb = np.zeros((pad_rows, D), dtype=bf16)
    xb[:rows] = x[r0:r1].astype(bf16)
    # (128, nt, D): partition p holds row t*128 + p
    x_nat = np.ascontiguousarray(xb.reshape(nt, 128, D).transpose(1, 0, 2))

    seg_local = np.full(pad_rows, -1, dtype=np.int64)
    seg_local[:rows] = batch[r0:r1] - seg0
    ci = np.where(seg_local < 0, -1.0, seg_local % EW).astype(np.float32)
    colidx = np.ascontiguousarray(ci.reshape(nt, 128).T).astype(bf16)

    sizes = _group_plan(nt)
    scat = np.zeros((EW, len(sizes), SEGS_PER_CORE), dtype=np.float32)
    gstart = 0
    for g, gsize in enumerate(sizes):
        segs = np.unique(seg_local[gstart * 128:(gstart + gsize) * 128])
        segs = segs[segs >= 0]
        assert segs.size <= EW, f"group {g} spans {segs.size} segments > EW"
        scat[segs % EW, g, segs] = 1.0
        gstart += gsize

    return {"x_nat": x_nat, "colidx": colidx, "scat": scat}


def _shared_inputs(W1, W2):
    w1t = np.ascontiguousarray(
        W1.T.astype(bf16).reshape(2, H, H).transpose(1, 0, 2))
    w2c = np.ascontiguousarray(W2.reshape(H, 1).astype(bf16))
    iota = np.broadcast_to(
        np.arange(EW, dtype=np.float32), (128, EW)).astype(bf16)
    ident = np.eye(128, dtype=bf16)
    return {"W1T": w1t, "W2c": w2c, "iota": iota, "ident": ident,
            "identf": np.eye(128, dtype=np.float32)}


def _seg_starts(x, batch):
    s = np.searchsorted(batch, np.arange(0, NCORES * SEGS_PER_CORE + 1,
                                         SEGS_PER_CORE))
    s[0], s[-1] = 0, x.shape[0]
    return s


def build_in_maps(x, batch, nt):
    s = _seg_starts(x, batch)
    return [_prep_core(x, batch, int(s[c]), int(s[c + 1]), c * SEGS_PER_CORE, nt)
            for c in range(NCORES)]


def pick_nt(x, batch):
    s = _seg_starts(x, batch)
    nt = int(max(-(-(int(s[c + 1] - s[c])) // 128) for c in range(NCORES)))
    return -(-nt // QUAD) * QUAD


def kernel(x, batch, W1, W2, B):
    x = np.asarray(x)
    batch = np.asarray(batch)
    W1 = np.asarray(W1)
    W2 = np.asarray(W2)
    B = int(B)
    assert B == NCORES * SEGS_PER_CORE

    nt = pick_nt(x, batch)
    if nt not in _kernel_cache:
        _kernel_cache.clear()
        _kernel_cache[nt] = _build_kernel(nt)
    nc = _kernel_cache[nt]

    shared = _shared_inputs(W1, W2)
    in_maps = build_in_maps(x, batch, nt)
    for m in in_maps:
        m.update(shared)

    res = run_bass_kernel_spmd(nc, in_maps, core_ids=list(range(NCORES)))

    seg_starts = _seg_starts(x, batch)
    z = np.empty((B, D), dtype=np.float32)
    for c in range(NCORES):
        num = res.results[c]["out"]  # (128, D)
        e = res.results[c]["e_out"].T.reshape(-1)
        r0, r1 = int(seg_starts[c]), int(seg_starts[c + 1])
        seg_local = (batch[r0:r1] - c * SEGS_PER_CORE).astype(np.int64)
        e_rows = e[:r1 - r0].astype(np.float64)
        den = np.bincount(seg_local, weights=e_rows, minlength=SEGS_PER_CORE)
        den = np.where(den == 0.0, 1.0, den).astype(np.float32)
        z[c * SEGS_PER_CORE:(c + 1) * SEGS_PER_CORE] = num / den[:, None]
    return z


# revision 3
# speedup vs baseline: 1.8358x; 1.0096x over previous
"""AttentionPooling v3: ship x once (natural bf16), transpose on-chip.

z[b] = sum_i softmax_within_segment(alpha)_i * x_i, alpha = tanh(x@W1.T)@W2.T.

vs v1 (which shipped x in BOTH orientations = 64MB/core, DMA-bound at
~200us): ship only x_nat (32MB/core), and build x^T on-chip:

- per 4-tile quad: PE-transposes x_nat 128x128 blocks into PSUM (bf16),
  then DVE/ACT copy them to SBUF (alternating, to balance engine load).
  mm1 (y^T = W1 @ x^T) consumes that SBUF copy exactly like v1.
- pooling is the v1 one-hot scheme but FLIPPED: gp^T[d, ew] += x_chunk^T @ E
  with x_nat as the stationary operand (out free size 32 instead of 256).
  Per group: gp^T -> (ACT evac) -> PE transpose -> gp[ew, d] -> f32 scatter
  matmul into the persistent per-segment pool (exact, as v1).
- exp batched per group (1 ACT op), E-build batched per group (2 DVE ops).
- host: denominators from the e_buf dump, z = pool / den (same as v1).

Engine budget per core (sim): DMA ~97us (32MB), PE ~135us (transposes 53
+ mm1 53 + pool 13 + misc), ACT ~95 + evac share, DVE ~37 + evac share.
"""

import numpy as np
import ml_dtypes

import concourse.bacc as bacc
import concourse.mybir as mybir
import concourse.tile as tile
from concourse.bass_utils import run_bass_kernel_spmd

bf16 = ml_dtypes.bfloat16
F32 = mybir.dt.float32
BF16 = mybir.dt.bfloat16
AF = mybir.ActivationFunctionType
ALU = mybir.AluOpType

NCORES = 8
D = 256
H = 128
SEGS_PER_CORE = 128
GT = 64          # tiles per group (a group's segments must fit mod-EW)
EW = 32
QUAD = 4         # tiles per mm1/psum batch
DMA_PIECE = 8    # tiles per xn DMA transfer (finer pieces pipeline better)

# every XBAR_EVERY-th quad is transposed by the DMA xbar instead of the PE
# (disabled: per-block issue overhead ~650ns dwarfs the 420ns PE saving, and
# the issuing engine's SEQ blocks on the transpose's data waits)
XBAR_EVERY = 10 ** 9
XBAR_PHASE = 3

_kernel_cache = {}


def _group_plan(nt):
    # small leading groups so exp/E-build/pool start early (shorter
    # dependency ladder during pipeline fill), then full GT groups, then
    # small trailing groups (shorter drain chain after the last tile)
    sizes = []
    left = nt
    for s in (8, 8, 16, 32):
        if left >= s + 16:
            sizes.append(s)
            left -= s
    while left >= GT + 16:
        sizes.append(GT)
        left -= GT
    for s in (32, 16, 8, 4):
        while left >= s and (s == 4 or left >= s + 4 or left == s):
            sizes.append(s)
            left -= s
    assert sum(sizes) == nt and left == 0, (sizes, nt)
    return sizes


def _build_kernel(nt):
    assert nt % QUAD == 0
    sizes = _group_plan(nt)
    ngroups = len(sizes)

    nc = bacc.Bacc("TRN2", target_bir_lowering=False, debug=False)

    xn_d = nc.dram_tensor("x_nat", [128, nt, D], BF16, kind="ExternalInput").ap()
    ci_d = nc.dram_tensor("colidx", [128, nt], BF16, kind="ExternalInput").ap()
    w1t_d = nc.dram_tensor("W1T", [128, 2, H], BF16, kind="ExternalInput").ap()
    w2_d = nc.dram_tensor("W2c", [H, 1], BF16, kind="ExternalInput").ap()
    iota_d = nc.dram_tensor("iota", [128, EW], BF16, kind="ExternalInput").ap()
    ident_d = nc.dram_tensor("ident", [128, 128], BF16, kind="ExternalInput").ap()
    identf_d = nc.dram_tensor("identf", [128, 128], F32, kind="ExternalInput").ap()
    scat_d = nc.dram_tensor("scat", [EW, ngroups, SEGS_PER_CORE], F32,
                            kind="ExternalInput").ap()
    out_d = nc.dram_tensor("out", [SEGS_PER_CORE, D], F32, kind="ExternalOutput").ap()
    e_out_d = nc.dram_tensor("e_out", [128, nt], BF16, kind="ExternalOutput").ap()

    with tile.TileContext(nc) as tc:
        with (
            tc.tile_pool(name="const", bufs=1) as constp,
            tc.tile_pool(name="xn", bufs=4) as xnp,
            tc.tile_pool(name="xt", bufs=6) as xtp,
            tc.tile_pool(name="th", bufs=4) as thp,
            tc.tile_pool(name="e4", bufs=2) as e4p,
            tc.tile_pool(name="gps", bufs=2) as gpsp,
            tc.tile_pool(name="out", bufs=1) as outp,
            tc.tile_pool(name="psum_y", bufs=2, space="PSUM") as psumy,
            tc.tile_pool(name="psum_xt", bufs=2, space="PSUM") as psumxt,
            tc.tile_pool(name="psum_al", bufs=1, space="PSUM") as psumal,
            tc.tile_pool(name="psum_gp", bufs=1, space="PSUM") as psumgp,
            tc.tile_pool(name="psum_gt", bufs=1, space="PSUM") as psumgt,
            tc.tile_pool(name="psum_acc", bufs=1, space="PSUM") as psumacc,
        ):
            # first group's x is loaded per-quad ahead of the other consts so
            # the PE can start transposing ~2us in instead of ~14us
            ident_sb = constp.tile([128, 128], BF16)
            nc.sync.dma_start(ident_sb[:], ident_d[:])
            g0size = sizes[0]
            xn0 = xnp.tile([128, g0size, D], BF16, tag="xn",
                           padded_shape=[128, GT, D])
            nc.sync.dma_start(xn0[:, 0:QUAD, :], xn_d[:, 0:QUAD, :])
            w1t_sb = constp.tile([128, 2, H], BF16)
            nc.sync.dma_start(w1t_sb[:], w1t_d[:])
            w2_sb = constp.tile([H, 1], BF16)
            nc.sync.dma_start(w2_sb[:], w2_d[:])
            for q0 in range(QUAD, g0size, QUAD):
                qn = min(QUAD, g0size - q0)
                nc.sync.dma_start(xn0[:, q0:q0 + qn, :],
                                  xn_d[:, q0:q0 + qn, :])
            iota_sb = constp.tile([128, EW], BF16)
            nc.sync.dma_start(iota_sb[:], iota_d[:])
            identf_sb = constp.tile([128, 128], F32)
            nc.sync.dma_start(identf_sb[:], identf_d[:])
            ci_sb = constp.tile([128, nt], BF16)
            nc.sync.dma_start(ci_sb[:], ci_d[:])
            scat_sb = constp.tile([EW, ngroups, SEGS_PER_CORE], F32)
            nc.sync.dma_start(scat_sb[:], scat_d[:])
            e_buf = constp.tile([128, nt], BF16)

            pool_ps = psumacc.tile([SEGS_PER_CORE, D], F32)

            # deferred pooling closures, run one group behind compute
            pending_pool = []

            def emit_pool(g, gsize, gstart, xn, e4g):
                gpt_ps = psumgt.tile([128, 2, EW], F32, tag="gpt")
                for c in range(2):
                    for tg in range(gsize):
                        nc.tensor.matmul(
                            gpt_ps[:, c, :],
                            xn[:, tg, c * 128:(c + 1) * 128],
                            e4g[:, tg, :],
                            start=(tg == 0), stop=(tg == gsize - 1))
                gpt_sb = gpsp.tile([128, 2, EW], F32, tag="gpt_sb")
                nc.scalar.activation(gpt_sb[:], gpt_ps[:], AF.Copy)
                gp_ps = psumgp.tile([EW, D], F32, tag="gp")
                for c in range(2):
                    nc.tensor.transpose(gp_ps[:, c * 128:(c + 1) * 128],
                                        gpt_sb[:, c, :], identf_sb[:])
                gp_sb = gpsp.tile([EW, D], F32, tag="gp_sb")
                nc.scalar.activation(gp_sb[:], gp_ps[:], AF.Copy)
                nc.tensor.matmul(pool_ps[:], scat_sb[:, g, :], gp_sb[:],
                                 start=(g == 0), stop=(g == ngroups - 1))

            gstart = 0
            evac_i = 0
            for g, gsize in enumerate(sizes):
                if g == 0:
                    xn = xn0
                else:
                    xn = xnp.tile([128, gsize, D], BF16, tag="xn",
                                  padded_shape=[128, GT, D])
                    for p0 in range(0, gsize, DMA_PIECE):
                        pn = min(DMA_PIECE, gsize - p0)
                        nc.sync.dma_start(
                            xn[:, p0:p0 + pn, :],
                            xn_d[:, gstart + p0:gstart + p0 + pn, :])

                al_ps = psumal.tile([128, gsize], F32, tag="al",
                                    padded_shape=[128, GT])

                for q0 in range(0, gsize, QUAD):
                    qn = min(QUAD, gsize - q0)
                    xt_sb = xtp.tile([128, 2, qn * 128], BF16, tag="xt_sb",
                                     padded_shape=[128, 2, QUAD * 128])
                    if False:
                        # xbar-DMA transpose straight to SBUF (no PSUM, no
                        # evac); issued on the ACT/DVE queues to keep the SP
                        # sequencer free
                        eng = nc.scalar if (evac_i // XBAR_EVERY) % 2 == 0 \
                            else nc.sync
                        for j in range(qn):
                            t = q0 + j
                            for c in range(2):
                                eng.dma_start_transpose(
                                    xt_sb[:, c, j * 128:(j + 1) * 128],
                                    xn[:, t, c * 128:(c + 1) * 128])
                    else:
                        # PE transpose into PSUM (bf16), evac via DVE
                        xt_ps = psumxt.tile([128, 2, qn * 128], BF16,
                                            tag="xt_ps",
                                            padded_shape=[128, 2, QUAD * 128])
                        for j in range(qn):
                            t = q0 + j
                            for c in range(2):
                                nc.tensor.transpose(
                                    xt_ps[:, c, j * 128:(j + 1) * 128],
                                    xn[:, t, c * 128:(c + 1) * 128],
                                    ident_sb[:])
                        if evac_i % 8 == 5:
                            nc.scalar.activation(xt_sb[:], xt_ps[:], AF.Copy)
                        else:
                            nc.vector.tensor_copy(xt_sb[:], xt_ps[:])
                    evac_i += 1

                    # mm1: y^T = W1c @ x^T  (accumulate over 2 chunks)
                    y_ps = psumy.tile([128, qn * 128], F32, tag="y",
                                      padded_shape=[128, QUAD * 128])
                    for c in range(2):
                        nc.tensor.matmul(y_ps[:], w1t_sb[:, c, :],
                                         xt_sb[:, c, :],
                                         start=(c == 0), stop=(c == 1))
                    th = thp.tile([128, qn * 128], BF16, tag="th",
                                  padded_shape=[128, QUAD * 128])
                    nc.scalar.activation(th[:], y_ps[:], AF.Tanh)
                    for j in range(qn):
                        nc.tensor.matmul(al_ps[:, q0 + j:q0 + j + 1],
                                         th[:, j * 128:(j + 1) * 128],
                                         w2_sb[:], start=True, stop=True)

                nc.scalar.activation(e_buf[:, gstart:gstart + gsize], al_ps[:],
                                     AF.Exp)

                # one-hot E (weighted by e) for the whole group: 2 DVE ops
                s4 = e4p.tile([128, gsize, EW], BF16, tag="s4",
                              padded_shape=[128, GT, EW])
                nc.vector.tensor_tensor(
                    s4[:],
                    ci_sb[:, gstart:gstart + gsize].broadcast_to(
                        [128, gsize, EW]),
                    iota_sb[:, None, :].broadcast_to([128, gsize, EW]),
                    ALU.is_equal)
                e4g = e4p.tile([128, gsize, EW], BF16, tag="e4g",
                               padded_shape=[128, GT, EW])
                nc.vector.tensor_mul(
                    e4g[:], s4[:],
                    e_buf[:, gstart:gstart + gsize].broadcast_to(
                        [128, gsize, EW]))

                # run the PREVIOUS group's pooling now (keeps PE from
                # stalling on this group's E4)
                for fn in pending_pool:
                    fn()
                pending_pool = [
                    (lambda g=g, gsize=gsize, gstart=gstart, xn=xn, e4g=e4g:
                     emit_pool(g, gsize, gstart, xn, e4g))]
                gstart += gsize

            for fn in pending_pool:
                fn()

            pool_sb = outp.tile([SEGS_PER_CORE, D], F32)
            nc.scalar.activation(pool_sb[:], pool_ps[:], AF.Copy)
            nc.sync.dma_start(out_d[:], pool_sb[:])
            nc.sync.dma_start(e_out_d[:], e_buf[:])

    nc.compile()
    return nc


def _prep_core(x, batch, r0, r1, seg0, nt):
    rows = r1 - r0
    pad_rows = nt * 128

    xb = np.zeros((pad_rows, D), dtype=bf16)
    xb[:rows] = x[r0:r1].astype(bf16)
    # (128, nt, D): partition p holds row t*128 + p
    x_nat = np.ascontiguousarray(xb.reshape(nt, 128, D).transpose(1, 0, 2))

    seg_local = np.full(pad_rows, -1, dtype=np.int64)
    seg_local[:rows] = batch[r0:r1] - seg0
    ci = np.where(seg_local < 0, -1.0, seg_local % EW).astype(np.float32)
    colidx = np.ascontiguousarray(ci.reshape(nt, 128).T).astype(bf16)

    sizes = _group_plan(nt)
    scat = np.zeros((EW, len(sizes), SEGS_PER_CORE), dtype=np.float32)
    gstart = 0
    for g, gsize in enumerate(sizes):
        segs = np.unique(seg_local[gstart * 128:(gstart + gsize) * 128])
        segs = segs[segs >= 0]
        assert segs.size <= EW, f"group {g} spans {segs.size} segments > EW"
        scat[segs % EW, g, segs] = 1.0
        gstart += gsize

    return {"x_nat": x_nat, "colidx": colidx, "scat": scat}


def _shared_inputs(W1, W2):
    w1t = np.ascontiguousarray(
        W1.T.astype(bf16).reshape(2, H, H).transpose(1, 0, 2))
    w2c = np.ascontiguousarray(W2.reshape(H, 1).astype(bf16))
    iota = np.broadcast_to(
        np.arange(EW, dtype=np.float32), (128, EW)).astype(bf16)
    ident = np.eye(128, dtype=bf16)
    return {"W1T": w1t, "W2c": w2c, "iota": iota, "ident": ident,
            "identf": np.eye(128, dtype=np.float32)}


def _seg_starts(x, batch):
    s = np.searchsorted(batch, np.arange(0, NCORES * SEGS_PER_CORE + 1,
                                         SEGS_PER_CORE))
    s[0], s[-1] = 0, x.shape[0]
    return s


def build_in_maps(x, batch, nt):
    s = _seg_starts(x, batch)
    return [_prep_core(x, batch, int(s[c]), int(s[c + 1]), c * SEGS_PER_CORE, nt)
            for c in range(NCORES)]


def pick_nt(x, batch):
    s = _seg_starts(x, batch)
    nt = int(max(-(-(int(s[c + 1] - s[c])) // 128) for c in range(NCORES)))
    return -(-nt // QUAD) * QUAD


def kernel(x, batch, W1, W2, B):
    x = np.asarray(x)
    batch = np.asarray(batch)
    W1 = np.asarray(W1)
    W2 = np.asarray(W2)
    B = int(B)
    assert B == NCORES * SEGS_PER_CORE

    nt = pick_nt(x, batch)
    if nt not in _kernel_cache:
        _kernel_cache.clear()
        _kernel_cache[nt] = _build_kernel(nt)
    nc = _kernel_cache[nt]

    shared = _shared_inputs(W1, W2)
    in_maps = build_in_maps(x, batch, nt)
    for m in in_maps:
        m.update(shared)

    res = run_bass_kernel_spmd(nc, in_maps, core_ids=list(range(NCORES)))

    seg_starts = _seg_starts(x, batch)
    z = np.empty((B, D), dtype=np.float32)
    for c in range(NCORES):
        num = res.results[c]["out"]  # (128, D)
        e = res.results[c]["e_out"].T.reshape(-1)
        r0, r1 = int(seg_starts[c]), int(seg_starts[c + 1])
        seg_local = (batch[r0:r1] - c * SEGS_PER_CORE).astype(np.int64)
        e_rows = e[:r1 - r0].astype(np.float64)
        den = np.bincount(seg_local, weights=e_rows, minlength=SEGS_PER_CORE)
        den = np.where(den == 0.0, 1.0, den).astype(np.float32)
        z[c * SEGS_PER_CORE:(c + 1) * SEGS_PER_CORE] = num / den[:, None]
    return z
